# revision 18
# baseline (speedup 1.0000x reference)
"""Multi-headed causal self-attention on 8 Trainium2 NeuronCores.

Sharding: tensor-parallel over heads (2 of 16 heads per core).
Each core computes Q/K/V projections for its 256-wide feature slice,
causal attention for its 2 heads, and the partial output projection
through its slice of Wo.  The host sums the 8 partial outputs (bf16)
and adds the analytically-folded constant row  bo + Wo @ bv.

On-chip layout (all matmul operands bf16):
  - X / weights are host-pre-tiled so every DMA reads contiguous
    multi-KB per-partition segments (full descriptor efficiency).
  - Q, K are produced feature-major [d_head, tok]; V token-major.
  - scores are computed transposed (scoresT [k, q]); softmax skips
    max-subtraction (logits bounded); causal masking adds -60 on
    diagonal blocks; blocks above the diagonal are never computed.
  - softmax denominators accumulate in bf16 on the GPSIMD engine; the
    denominator matmul + reciprocal + normalize of chunk c are DEFERRED
    into chunk c+1's projection phase so they never stall the PE.
  - score(kt+1) is emitted ahead of AV(kt) so the AV matmul's exp
    dependency is already satisfied when the PE reaches it.
  - out-projection of chunk c-1 is interleaved into chunk c's
    attention; output written bf16, packed [128, 2048] per row-block.
"""

import ml_dtypes
import numpy as np

import concourse.bass as bass  # noqa: F401  (registers engine types)
import concourse.tile as tile
from concourse import bacc, mybir
from concourse.bass_utils import run_bass_kernel_spmd


N_CORES = 8
B, S, D = 2, 2048, 2048
H, DH = 16, 128
HPC = H // N_CORES          # heads per core
DSH = HPC * DH              # per-core feature slice width (256)
TOK = B * S
P = 128                     # SBUF partitions
QCW = 512                   # query-chunk width (matmul moving dim)
NQC = S // QCW              # q-chunks per batch
NKT = S // P                # k-tiles per batch
KTPC = QCW // P             # k-tiles per q-chunk
NJC = D // QCW              # output column chunks
NCH = B * NQC               # total token chunks
SCALE = float(1.0 / np.sqrt(np.sqrt(DH)))
MASK_NEG = -60.0

F32 = mybir.dt.float32
BF16 = mybir.dt.bfloat16
MMD = BF16                  # matmul operand dtype
AFT = mybir.ActivationFunctionType

TRACE = False
LAST = {}

_nc = None


def _emit(tc, t):
    from contextlib import ExitStack

    nc = tc.nc
    with ExitStack() as ctx:
        const = ctx.enter_context(tc.tile_pool(name="const", bufs=1))
        xtp = ctx.enter_context(tc.tile_pool(name="xtp", bufs=3))
        kvp = ctx.enter_context(tc.tile_pool(name="kvp", bufs=2))
        qch = ctx.enter_context(tc.tile_pool(name="qch", bufs=2))
        ach = ctx.enter_context(tc.tile_pool(name="ach", bufs=2))
        expp = ctx.enter_context(tc.tile_pool(name="expp", bufs=8))
        sacp = ctx.enter_context(tc.tile_pool(name="sacp", bufs=4))
        smlp = ctx.enter_context(tc.tile_pool(name="smlp", bufs=2))
        outsp = ctx.enter_context(tc.tile_pool(name="outsp", bufs=4))
        psA = ctx.enter_context(tc.tile_pool(name="psA", bufs=2, space="PSUM"))
        psS = ctx.enter_context(tc.tile_pool(name="psS", bufs=3, space="PSUM"))
        psT = ctx.enter_context(tc.tile_pool(name="psT", bufs=2, space="PSUM"))
        psD = ctx.enter_context(tc.tile_pool(name="psD", bufs=1, space="PSUM"))

        # ---- constants ----
        # scalar HWDGE ring: wq; sync ring: X chunks + wv + wo;
        # gpsimd SWDGE ring: biases + wk + small constants
        wq_sb = const.tile([P, NKT, DSH], MMD)
        nc.scalar.dma_start(out=wq_sb[:, 0:2, :], in_=t["wqt"][:, 0:2, :])
        nc.scalar.dma_start(out=wq_sb[:, 2:8, :], in_=t["wqt"][:, 2:8, :])
        nc.scalar.dma_start(out=wq_sb[:, 8:16, :], in_=t["wqt"][:, 8:16, :])
        wk_sb = const.tile([P, NKT, DSH], MMD)
        wv_sb = const.tile([P, NKT, DSH], MMD)
        wo_sb = const.tile([P, HPC, D], MMD)
        bq_sb = const.tile([P, HPC, 1], F32)
        nc.gpsimd.dma_start(out=bq_sb, in_=t["bqs"])
        bk_sb = const.tile([P, HPC, 1], F32)
        nc.gpsimd.dma_start(out=bk_sb, in_=t["bks"])
        nc.gpsimd.dma_start(out=wk_sb, in_=t["wkt"])
        tri_sb = const.tile([P, P], F32)
        nc.gpsimd.dma_start(out=tri_sb, in_=t["tri"])
        ones_mat = const.tile([P, P], MMD)
        nc.gpsimd.dma_start(out=ones_mat, in_=t["onesc"])

        xt_view = t["xt"]   # [P, NCH, NKT, QCW] bf16, contiguous per partition

        def outproj_unit(bp, qcp, a_prev, ot, tt, jc, dmae=None):
            # one [128-token x 512-col] slab of the previous chunk's
            # out-projection; woven between attention blocks to keep PE fed
            po = psA.tile([P, QCW], F32, tag="ps")
            for h in range(HPC):
                nc.tensor.matmul(po, a_prev[:, h, tt * P:(tt + 1) * P],
                                 wo_sb[:, h, jc * QCW:(jc + 1) * QCW],
                                 start=(h == 0), stop=(h == HPC - 1))
            if (tt + jc) % 2 == 0:
                nc.vector.tensor_copy(ot[:, jc * QCW:(jc + 1) * QCW], po)
            else:
                nc.scalar.copy(ot[:, jc * QCW:(jc + 1) * QCW], po)
            if jc == NJC - 1:
                row0 = bp * S + (qcp * KTPC + tt) * P
                (dmae or nc.sync).dma_start(
                    out=t["outp"][row0:row0 + P, :], in_=ot)

        def finish_one(sacc, at, dst):
            # deferred softmax finish: denominator matmul, reciprocal,
            # normalize into the bf16 a tile consumed by the out-projection
            dnb = psD.tile([P, QCW], F32, tag="dn")
            nc.tensor.matmul(dnb, ones_mat, sacc, start=True, stop=True)
            rcf = smlp.tile([P, QCW], F32, tag="rcf")
            nc.vector.reciprocal_approx_fast(rcf, dnb)
            nc.vector.tensor_mul(dst, at, rcf)

        pending = None   # (b, qc, [sacc0, sacc1], [at0, at1]) awaiting finish
        aprev = None     # (b, qc, a_sb) feeding interleaved out-projection
        for b in range(B):
            k_sb = kvp.tile([P, HPC, S], MMD, tag="k")
            v_sb = kvp.tile([P, NKT, DSH], MMD, tag="v")
            for qc in range(NQC):
                c = b * NQC + qc

                # ---- P1: QKV projections (+ deferred softmax finish) ----
                xt0 = xtp.tile([P, NKT, QCW], MMD, tag="xt")
                if c == 0:
                    nc.sync.dma_start(out=xt0[:, 0:2, :], in_=xt_view[:, 0, 0:2, :])
                    nc.sync.dma_start(out=xt0[:, 2:8, :], in_=xt_view[:, 0, 2:8, :])
                    nc.sync.dma_start(out=xt0[:, 8:16, :], in_=xt_view[:, 0, 8:16, :])
                    # wv / wo ride the sync ring behind the first X chunk
                    nc.sync.dma_start(out=wv_sb, in_=t["wvt"])
                    nc.sync.dma_start(out=wo_sb, in_=t["wot"])
                else:
                    nc.sync.dma_start(out=xt0, in_=xt_view[:, c, :, :])
                if c + 1 < NCH:
                    xt1 = xtp.tile([P, NKT, QCW], MMD, tag="xt")
                    nc.sync.dma_start(out=xt1, in_=xt_view[:, c + 1, :, :])

                if pending is not None:
                    a_new = ach.tile([P, HPC, QCW], MMD, tag="a")
                    finish_one(pending[2][0], pending[3][0], a_new[:, 0, :])
                else:
                    a_new = None
                q_sb = qch.tile([P, HPC, QCW], MMD, tag="q")
                if c == 0:
                    # k-pass order so compute starts after the first small
                    # DMA pieces land
                    qps = [psA.tile([P, QCW], F32, tag="ps", name="qp")
                           for _ in range(HPC)]
                    for k0, k1 in ((0, 2), (2, 8), (8, 16)):
                        for j in range(HPC):
                            for k in range(k0, k1):
                                nc.tensor.matmul(
                                    qps[j], wq_sb[:, k, j * DH:(j + 1) * DH],
                                    xt0[:, k, :],
                                    start=(k == 0), stop=(k == NKT - 1))
                    for j in range(HPC):
                        nc.scalar.activation(q_sb[:, j, :], qps[j], AFT.Identity,
                                             bias=bq_sb[:, j, :], scale=SCALE)
                else:
                    for j in range(HPC):
                        qp = psA.tile([P, QCW], F32, tag="ps")
                        for k in range(NKT):
                            nc.tensor.matmul(
                                qp, wq_sb[:, k, j * DH:(j + 1) * DH],
                                xt0[:, k, :],
                                start=(k == 0), stop=(k == NKT - 1))
                        nc.scalar.activation(q_sb[:, j, :], qp, AFT.Identity,
                                             bias=bq_sb[:, j, :], scale=SCALE)
                        if j == 0 and pending is not None:
                            finish_one(pending[2][1], pending[3][1],
                                       a_new[:, 1, :])
                            aprev = (pending[0], pending[1], a_new)
                            pending = None
                for j in range(HPC):
                    kp = psA.tile([P, QCW], F32, tag="ps")
                    for k in range(NKT):
                        nc.tensor.matmul(
                            kp, wk_sb[:, k, j * DH:(j + 1) * DH],
                            xt0[:, k, :],
                            start=(k == 0), stop=(k == NKT - 1))
                    nc.scalar.activation(k_sb[:, j, qc * QCW:(qc + 1) * QCW], kp,
                                         AFT.Identity, bias=bk_sb[:, j, :], scale=SCALE)
                for tt in range(KTPC):
                    vp = psA.tile([P, QCW], F32, tag="ps")
                    for k in range(NKT):
                        nc.tensor.matmul(
                            vp[:, 0:DSH],
                            xt0[:, k, tt * P:(tt + 1) * P],
                            wv_sb[:, k, :],
                            start=(k == 0), stop=(k == NKT - 1))
                    nc.vector.tensor_copy(v_sb[:, qc * KTPC + tt, :], vp[:, 0:DSH])

                # ---- P2: causal attention; score(kt+1) runs ahead of
                # AV(kt); previous chunk's out-projection interleaved ----
                nkt_q = (qc + 1) * KTPC
                units = ([(tt, jc) for tt in range(KTPC) for jc in range(NJC)]
                         if aprev is not None else [])
                ui = 0
                ots = {}

                def fire_unit():
                    nonlocal ui
                    if ui < len(units):
                        tt, jc = units[ui]
                        if jc == 0:
                            ots[tt] = outsp.tile([P, D], MMD, tag="ot",
                                                 name="ot")
                        outproj_unit(aprev[0], aprev[1], aprev[2],
                                     ots[tt], tt, jc)
                        ui += 1

                saccs, ats = [], []
                for h in range(HPC):
                    sacc = sacp.tile([P, QCW], MMD, tag="sacc")
                    at = psT.tile([P, QCW], F32, tag="at")
                    ets = [None] * nkt_q
                    offs = [max(kt - qc * KTPC, 0) * P for kt in range(nkt_q)]

                    def emit_score(kt):
                        off = offs[kt]
                        w = QCW - off
                        sp = psS.tile([P, QCW], F32, tag="sc")
                        nc.tensor.matmul(
                            sp[:, 0:w], k_sb[:, h, kt * P:(kt + 1) * P],
                            q_sb[:, h, off:QCW], start=True, stop=True)
                        if kt - qc * KTPC >= 0:
                            nc.vector.tensor_add(sp[:, 0:P], sp[:, 0:P], tri_sb)
                        et = expp.tile([P, QCW], MMD, tag="exp")
                        nc.scalar.activation(et[:, 0:w], sp[:, 0:w], AFT.Exp)
                        if kt == 0:
                            nc.vector.tensor_copy(sacc, et)
                        else:
                            nc.vector.tensor_add(sacc[:, off:QCW],
                                                 sacc[:, off:QCW], et[:, 0:w])
                        ets[kt] = et

                    def emit_av(kt):
                        off = offs[kt]
                        w = QCW - off
                        nc.tensor.matmul(
                            at[:, off:QCW], v_sb[:, kt, h * DH:(h + 1) * DH],
                            ets[kt][:, 0:w], start=(kt == 0),
                            stop=(kt == nkt_q - 1))

                    emit_score(0)
                    for kt in range(1, nkt_q):
                        emit_score(kt)
                        emit_av(kt - 1)
                        fire_unit()
                    emit_av(nkt_q - 1)
                    fire_unit()
                    saccs.append(sacc)
                    ats.append(at)
                    if c == NCH - 1 and h == 0:
                        # last chunk: finish head 0 during head 1's attention
                        a_fl = ach.tile([P, HPC, QCW], MMD, tag="a")
                        finish_one(sacc, at, a_fl[:, 0, :])
                while ui < len(units):
                    fire_unit()
                pending = (b, qc, saccs, ats)

        # ---- flush: finish chunk NCH-1 head 1 and emit its out-proj ----
        finish_one(pending[2][1], pending[3][1], a_fl[:, 1, :])
        aprev = (pending[0], pending[1], a_fl)
        for tt in range(KTPC):
            ot = outsp.tile([P, D], MMD, tag="ot")
            for jc in range(NJC):
                outproj_unit(aprev[0], aprev[1], aprev[2], ot, tt, jc,
                             dmae=(nc.scalar if tt % 2 else nc.sync))


def _build():
    nc = bacc.Bacc("TRN2", target_bir_lowering=False, debug=False,
                   num_devices=N_CORES)
    t = {
        "xt": nc.dram_tensor("xt", [P, NCH, NKT, QCW], BF16,
                             kind="ExternalInput").ap(),
        "wqt": nc.dram_tensor("wqt", [P, NKT, DSH], BF16,
                              kind="ExternalInput").ap(),
        "wkt": nc.dram_tensor("wkt", [P, NKT, DSH], BF16,
                              kind="ExternalInput").ap(),
        "wvt": nc.dram_tensor("wvt", [P, NKT, DSH], BF16,
                              kind="ExternalInput").ap(),
        "wot": nc.dram_tensor("wot", [P, HPC, D], BF16,
                              kind="ExternalInput").ap(),
        "bqs": nc.dram_tensor("bqs", [P, HPC, 1], F32, kind="ExternalInput").ap(),
        "bks": nc.dram_tensor("bks", [P, HPC, 1], F32, kind="ExternalInput").ap(),
        "tri": nc.dram_tensor("tri", [P, P], F32, kind="ExternalInput").ap(),
        "onesc": nc.dram_tensor("onesc", [P, P], BF16, kind="ExternalInput").ap(),
        "outp": nc.dram_tensor("outp", [TOK, D], BF16, kind="ExternalOutput").ap(),
    }
    with tile.TileContext(nc) as tc:
        _emit(tc, t)
    nc.compile()
    return nc


def _program():
    global _nc
    if _nc is None:
        _nc = _build()
    return _nc


def _tile_kd(w):
    # [D, M] -> [P, NKT, M] with [p, k, m] = w[k*128 + p, m], contiguous
    return np.ascontiguousarray(
        w.reshape(NKT, P, w.shape[1]).transpose(1, 0, 2)).astype(ml_dtypes.bfloat16)


def kernel(X, Wq, bq, Wk, bk, Wv, bv, Wo, bo):
    X = np.asarray(X, np.float32)
    Wq = np.asarray(Wq, np.float32)
    Wk = np.asarray(Wk, np.float32)
    Wv = np.asarray(Wv, np.float32)
    Wo = np.asarray(Wo, np.float32)
    bq = np.asarray(bq, np.float32)
    bk = np.asarray(bk, np.float32)
    bv = np.asarray(bv, np.float32)
    bo = np.asarray(bo, np.float32)

    nc = _program()

    # X^T tiled [P, NCH, NKT, QCW]: [p, c, k, q] = X^T[k*128+p, c*512+q]
    XT = X.reshape(TOK, D).T.astype(ml_dtypes.bfloat16)
    xt_t = np.ascontiguousarray(
        XT.reshape(NKT, P, NCH, QCW).transpose(1, 2, 0, 3))
    tri = np.where(np.arange(P)[:, None] <= np.arange(P)[None, :],
                   np.float32(0.0), np.float32(MASK_NEG)).astype(np.float32)
    ones_col = np.ones((P, P), ml_dtypes.bfloat16)

    in_maps = []
    for cidx in range(N_CORES):
        J = slice(cidx * DSH, (cidx + 1) * DSH)
        wot = np.ascontiguousarray(
            Wo[:, J].T.reshape(HPC, P, D).transpose(1, 0, 2)
        ).astype(ml_dtypes.bfloat16)
        in_maps.append({
            "xt": xt_t,
            "wqt": _tile_kd(np.ascontiguousarray(Wq[J, :].T)),
            "wkt": _tile_kd(np.ascontiguousarray(Wk[J, :].T)),
            "wvt": _tile_kd(np.ascontiguousarray(Wv[J, :].T)),
            "wot": wot,
            "bqs": np.ascontiguousarray(
                (bq[J] * SCALE).reshape(HPC, P).T).reshape(P, HPC, 1),
            "bks": np.ascontiguousarray(
                (bk[J] * SCALE).reshape(HPC, P).T).reshape(P, HPC, 1),
            "tri": tri,
            "onesc": ones_col,
        })

    res = run_bass_kernel_spmd(nc, in_maps, list(range(N_CORES)), trace=TRACE)
    LAST["res"] = res

    out = res.results[0]["outp"].astype(np.float32)
    for cidx in range(1, N_CORES):
        out += res.results[cidx]["outp"].astype(np.float32)
    out += (bo + Wo @ bv)[None, :].astype(np.float32)
    return out.reshape(B, S, D).astype(np.float32)
